# revision 1
# baseline (speedup 1.0000x reference)
"""Gaussian-KDE logsumexp kernel for Trainium2 (8 NeuronCores, SPMD).

Math: out[t] = logsumexp_n( -0.5 * scale[n] * dist2[t, n] ) - Z
with dist2 via the GEMM expansion. Everything folds into ONE matmul by
augmenting the contraction dim:
    xhat[:, t] = [test_t (64), test_sq_t, 1]                       (K = 66)
    yhat[:, n] = [scale_n*train_n (64), -.5*scale_n, -.5*scale_n*train_sq_n - Z]
so  xhat[:, t] . yhat[:, n] = -0.5*scale_n*dist2[t, n] - Z  = exp-argument.
Since weights ~ U[0,1], max_n over the exp-argument is within 1e-5 of -Z for
every t, so exp() never overflows and no per-row max pass is needed; the sum
lands ~e^-67 which is comfortably inside fp32 normal range.

Sharding: test points split 8 ways (256/core); train side replicated.
Per core: 2 t-tiles x [66,128] lhsT vs streamed yhat [66,4096];
ScalarE does exp with free-dim accumulation straight out of PSUM, then ln.
"""

import math
from contextlib import ExitStack

import numpy as np

import concourse.bacc as bacc
import concourse.bass as bass
import concourse.mybir as mybir
import concourse.tile as tile
from concourse.bass_utils import run_bass_kernel_spmd

N_CORES = 8
NT, NTR, D = 2048, 4096, 64
TPC = NT // N_CORES          # 256 test points per core
KA = D + 2                   # 66: augmented contraction dim
P = 128                      # partition tile of test points
T_TILES = TPC // P           # 2
MM_N = 512                   # matmul free-dim (one PSUM bank, fp32)
NB = NTR // MM_N             # 8 rhs blocks resident in SBUF
CHUNK = 2048                 # one ACT exp+accum instruction (4 PSUM banks)
N_CHUNKS = NTR // CHUNK      # 2
F32 = mybir.dt.float32
Z_CONST = float(0.5 * D * math.log(2.0 * math.pi) + math.log(NTR))  # h = 1

# float32r streams 1 col/cycle on the PE (vs 4 for float32); precision is
# validated against the fp32 path in test.py.
MM_DTYPE = mybir.dt.float32r


def build_program(mm_dtype=MM_DTYPE):
    nc = bacc.Bacc("TRN2")
    xh = nc.declare_dram_parameter("xhat", [KA, TPC], mm_dtype, isOutput=False)
    yh = nc.declare_dram_parameter("yhat", [KA, NTR], mm_dtype, isOutput=False)
    out_d = nc.declare_dram_parameter("out", [TPC], F32, isOutput=True)
    out_ap = out_d[:].rearrange("(a b) -> a b", b=1)

    with ExitStack() as ctx:
        tc = ctx.enter_context(tile.TileContext(nc))
        xpool = ctx.enter_context(tc.tile_pool(name="xpool", bufs=1))
        ypool = ctx.enter_context(tc.tile_pool(name="ypool", bufs=1))
        epool = ctx.enter_context(tc.tile_pool(name="epool", bufs=2))
        spool = ctx.enter_context(tc.tile_pool(name="spool", bufs=2))
        psum = ctx.enter_context(tc.tile_pool(name="psum", bufs=2, space="PSUM"))

        negz = xpool.tile([P, 1], F32, tag="negz")
        nc.vector.memset(negz, -Z_CONST)

        xs = xpool.tile([KA, TPC], mm_dtype, tag="xs")
        nc.sync.dma_start(out=xs, in_=xh[:])
        ys = []
        for h in range(N_CHUNKS):
            yt = ypool.tile([KA, CHUNK], mm_dtype, tag=f"y{h}")
            nc.sync.dma_start(out=yt, in_=yh[:, h * CHUNK:(h + 1) * CHUNK])
            ys.append(yt)

        tots = spool.tile([P, T_TILES], F32, tag="tots")
        for it in range(T_TILES):
            sums = spool.tile([P, N_CHUNKS], F32, tag="sums")
            for c in range(N_CHUNKS):
                pt = psum.tile([P, CHUNK], F32, tag="pt")
                for j in range(CHUNK // MM_N):
                    nc.tensor.matmul(
                        pt[:, j * MM_N:(j + 1) * MM_N],
                        xs[:, it * P:(it + 1) * P],
                        ys[c][:, j * MM_N:(j + 1) * MM_N],
                        start=True,
                        stop=True,
                    )
                et = epool.tile([P, CHUNK], F32, tag="et")
                nc.scalar.activation(
                    out=et,
                    in_=pt,
                    func=mybir.ActivationFunctionType.Exp,
                    accum_out=sums[:, c:c + 1],
                )
            nc.vector.reduce_sum(
                out=tots[:, it:it + 1], in_=sums, axis=mybir.AxisListType.X
            )
        # single Ln-table load for everything, then affine -Z (Identity is in
        # the same table set), then per-t-tile stores
        lnt = spool.tile([P, T_TILES], F32, tag="lnt")
        nc.scalar.activation(
            out=lnt, in_=tots, func=mybir.ActivationFunctionType.Ln
        )
        res = spool.tile([P, T_TILES], F32, tag="res")
        nc.scalar.activation(
            out=res,
            in_=lnt,
            func=mybir.ActivationFunctionType.Identity,
            bias=negz,
        )
        for it in range(T_TILES):
            nc.sync.dma_start(
                out=out_ap[it * P:(it + 1) * P, :], in_=res[:, it:it + 1]
            )
    nc.compile()
    return nc


def build_program_raw(mm_dtype=MM_DTYPE):
    """Hand-scheduled raw-Bass version: minimal semaphores, input DMAs and
    the ACT exp-table load hoisted ahead of the boot barrier, no Tile
    drain/barrier tail (the Sync engine finishes last by construction and
    clears the semaphores itself)."""
    nc = bacc.Bacc("TRN2")
    xh = nc.declare_dram_parameter("xhat", [KA, TPC], mm_dtype, isOutput=False)
    yh = nc.declare_dram_parameter("yhat", [KA, NTR], mm_dtype, isOutput=False)
    # [p, it] layout — contiguous DMA from res; host transposes when unsharding
    out_d = nc.declare_dram_parameter("out", [P, T_TILES], F32, isOutput=True)

    YB = 1024                    # columns per y DMA block
    NYB = NTR // YB              # 4
    G = T_TILES * N_CHUNKS       # 4 global chunks

    with ExitStack() as ctx:
        sb = lambda nm, shape, dt: ctx.enter_context(nc.sbuf_tensor(nm, shape, dt))
        xs = sb("xs", [KA, TPC], mm_dtype)
        ys = [sb(f"ys{b}", [KA, YB], mm_dtype) for b in range(NYB)]
        et = [sb(f"et{k}", [P, CHUNK], F32) for k in range(2)]
        dummy_in = sb("dummy_in", [P, 1], F32)
        dummy_out = sb("dummy_out", [P, 1], F32)
        zero = sb("zero", [P, 1], F32)
        negz = sb("negz", [P, 1], F32)
        sums = sb("sums", [P, G], F32)
        tots = sb("tots", [P, T_TILES], F32)
        lnt = sb("lnt", [P, T_TILES], F32)
        res = sb("res", [P, T_TILES], F32)
        pt = [
            ctx.enter_context(nc.psum_tensor(f"pt{k}", [P, CHUNK], F32))
            for k in range(2)
        ]

        sx = ctx.enter_context(nc.semaphore("sx"))
        sy = [ctx.enter_context(nc.semaphore(f"sy{b}")) for b in range(NYB)]
        spe = ctx.enter_context(nc.semaphore("spe"))
        sact = ctx.enter_context(nc.semaphore("sact"))
        svz = ctx.enter_context(nc.semaphore("svz"))
        sv2 = ctx.enter_context(nc.semaphore("sv2"))
        so = ctx.enter_context(nc.semaphore("so"))
        my_sems = [sx, *sy, spe, sact, svz, sv2, so]

        # Sync: input DMAs, issued immediately at boot. Stagger the y blocks
        # in two waves so the first chunk's data gets the full (66-partition
        # port-limited) bandwidth instead of fair-sharing with the later
        # blocks — the first matmuls start ~3us earlier.
        nc.sync.dma_start(out=xs[:], in_=xh[:]).then_inc(sx, 16)
        for b in range(2):
            nc.sync.dma_start(
                out=ys[b][:], in_=yh[:, b * YB:(b + 1) * YB]
            ).then_inc(sy[b], 16)
        nc.sync.wait_ge(sy[0], 16)
        nc.sync.wait_ge(sy[1], 16)
        for b in range(2, NYB):
            nc.sync.dma_start(
                out=ys[b][:], in_=yh[:, b * YB:(b + 1) * YB]
            ).then_inc(sy[b], 16)

        # ACT: trigger the (patched, exp+ln+identity) table load at boot;
        # bias AP is uninitialized garbage, output unused
        nc.scalar.activation(
            dummy_out[:],
            dummy_in[:],
            mybir.ActivationFunctionType.Exp,
            bias=zero[:],
        )

        # DVE: constants
        nc.vector.memset(zero[:], 0.0).then_inc(svz, 1)
        nc.vector.memset(negz[:], -Z_CONST).then_inc(svz, 1)

        # PE stream
        for g in range(G):
            it, c = divmod(g, N_CHUNKS)
            for j in range(CHUNK // MM_N):
                if j == 0 and g >= 2:
                    nc.tensor.wait_ge(sact, g - 1)  # PSUM buf recycled
                if j == 0 and g == 0:
                    nc.tensor.wait_ge(sx, 16)
                if j % 2 == 0:
                    nc.tensor.wait_ge(sy[2 * c + j // 2], 16)
                mm = nc.tensor.matmul(
                    pt[g % 2][:, j * MM_N:(j + 1) * MM_N],
                    xs[:, it * P:(it + 1) * P],
                    ys[2 * c + j // 2][:, (j % 2) * MM_N:(j % 2 + 1) * MM_N],
                    start=True,
                    stop=True,
                )
            mm.then_inc(spe, 1)

        # ACT stream: exp+accumulate per chunk, then ln, -Z, and the result
        # DMAs (ACT is an HWDGE engine, and Sync instructions are slow)
        nc.scalar.wait_ge(svz, 2)
        for g in range(G):
            nc.scalar.wait_ge(spe, g + 1)
            nc.scalar.activation(
                out=et[g % 2][:],
                in_=pt[g % 2][:],
                func=mybir.ActivationFunctionType.Exp,
                bias=zero[:],
                accum_out=sums[:, g:g + 1],
            ).then_inc(sact, 1)

        # DVE: per-t-tile totals
        for it in range(T_TILES):
            nc.vector.wait_ge(sact, N_CHUNKS * (it + 1))
            r = nc.vector.reduce_sum(
                out=tots[:, it:it + 1],
                in_=sums[:, it * N_CHUNKS:(it + 1) * N_CHUNKS],
                axis=mybir.AxisListType.X,
            )
        r.then_inc(sv2, 1)

        nc.scalar.wait_ge(sv2, 1)
        nc.scalar.activation(
            out=lnt[:],
            in_=tots[:],
            func=mybir.ActivationFunctionType.Ln,
            bias=zero[:],
        )
        nc.scalar.activation(
            out=res[:],
            in_=lnt[:],
            func=mybir.ActivationFunctionType.Identity,
            bias=negz[:],
        )
        # single result DMA; out[p, it] = res[p, it]
        nc.scalar.dma_start(out=out_d[:], in_=res[:]).then_inc(so, 16)

        # DVE: wait for the result DMA, then clear all our semaphores in
        # one ranged instruction (all other engines are past their final
        # waits once the out-DMA has completed)
        nc.vector.wait_ge(so, 16)
        sem_nums = sorted(s.num for s in my_sems)
        assert sem_nums == list(range(sem_nums[0], sem_nums[0] + len(sem_nums)))
        nc.vector.sem_clear(range(sem_nums[0], sem_nums[-1] + 1))

    nc.compile()
    # Post-compile surgery: collapse the two ACT table loads into a single
    # load of set 6 (natural_log_exp_and_others: exp + ln + identity), and
    # drop the constructor's const-AP memsets + all-engine boot barrier
    # (nothing reads the const APs; every engine can start immediately).
    _strip_preamble_and_merge_act_tables(nc)
    return nc


def build_program_packed(mm_dtype=MM_DTYPE):
    """Like build_program_raw, but the train-side matrix is transferred at
    full DMA port width: chunk0 (n 0..2047) lands natively as [66, 2048] on
    partitions 0-65, chunk1 (n 2048..4095) lands as a [64, 2048] feature
    block on partitions 64-127 (disjoint ports -> both transfer in
    parallel) plus a tiny [2, 2048] augmented-row block at partitions
    32-33. Chunk1 matmuls are split into a K=64 feat matmul (base 64) and
    a K=2 aug matmul (base 32) accumulating into the same PSUM block."""
    nc = bacc.Bacc("TRN2")
    ya_d = nc.declare_dram_parameter("ya", [KA, CHUNK], mm_dtype, isOutput=False)
    yb_d = nc.declare_dram_parameter("yb", [D, CHUNK], mm_dtype, isOutput=False)
    yba_d = nc.declare_dram_parameter("yba", [2, CHUNK], mm_dtype, isOutput=False)
    xa_d = nc.declare_dram_parameter("xa", [KA, TPC], mm_dtype, isOutput=False)
    xb_d = nc.declare_dram_parameter("xb", [D, TPC], mm_dtype, isOutput=False)
    xba_d = nc.declare_dram_parameter("xba", [2, TPC], mm_dtype, isOutput=False)
    out_d = nc.declare_dram_parameter("out", [P, T_TILES], F32, isOutput=True)

    G = T_TILES * N_CHUNKS       # 4 global chunks

    with ExitStack() as ctx:
        sb = lambda nm, shape, dt: ctx.enter_context(nc.sbuf_tensor(nm, shape, dt))
        ya = sb("ya_s", [KA, CHUNK], mm_dtype)
        yb = sb("yb_s", [P, CHUNK], mm_dtype)       # rows 64..127 used
        yba = sb("yba_s", [34, CHUNK], mm_dtype)    # rows 32..33 used
        xa = sb("xa_s", [KA, TPC], mm_dtype)
        xb = sb("xb_s", [P, TPC], mm_dtype)         # rows 64..127 used
        xba = sb("xba_s", [34, TPC], mm_dtype)      # rows 32..33 used
        et = [sb(f"et{k}", [P, CHUNK], F32) for k in range(2)]
        dummy_in = sb("dummy_in", [P, 1], F32)
        dummy_out = sb("dummy_out", [P, 1], F32)
        zero = sb("zero", [P, 1], F32)
        negz = sb("negz", [P, 1], F32)
        sums = sb("sums", [P, G], F32)
        tots = sb("tots", [P, T_TILES], F32)
        lnt = sb("lnt", [P, T_TILES], F32)
        res = sb("res", [P, T_TILES], F32)
        pt = [
            ctx.enter_context(nc.psum_tensor(f"pt{k}", [P, CHUNK], F32))
            for k in range(2)
        ]

        sya = ctx.enter_context(nc.semaphore("sya"))
        syb = ctx.enter_context(nc.semaphore("syb"))
        syba = ctx.enter_context(nc.semaphore("syba"))
        sxa = ctx.enter_context(nc.semaphore("sxa"))
        sxb = ctx.enter_context(nc.semaphore("sxb"))
        sxba = ctx.enter_context(nc.semaphore("sxba"))
        spe = ctx.enter_context(nc.semaphore("spe"))
        sact = ctx.enter_context(nc.semaphore("sact"))
        svz = ctx.enter_context(nc.semaphore("svz"))
        sv2 = ctx.enter_context(nc.semaphore("sv2"))
        so = ctx.enter_context(nc.semaphore("so"))
        my_sems = [sya, syb, syba, sxa, sxb, sxba, spe, sact, svz, sv2, so]

        # ACT: the critical chunk0 transfer goes first on the ACT queue so it
        # starts at engine boot, before the table load
        nc.scalar.dma_start(out=ya[:], in_=ya_d[:]).then_inc(sya, 16)
        # dummy exp triggers the (patched, exp+ln+identity) table load
        nc.scalar.activation(
            dummy_out[:],
            dummy_in[:],
            mybir.ActivationFunctionType.Exp,
            bias=zero[:],
        )

        # Sync: everything else
        nc.sync.dma_start(out=xa[:], in_=xa_d[:]).then_inc(sxa, 16)
        nc.sync.dma_start(out=yb[D:P, :], in_=yb_d[:]).then_inc(syb, 16)
        nc.sync.dma_start(out=xb[D:P, :], in_=xb_d[:]).then_inc(sxb, 16)
        nc.sync.dma_start(out=yba[32:34, :], in_=yba_d[:]).then_inc(syba, 16)
        nc.sync.dma_start(out=xba[32:34, :], in_=xba_d[:]).then_inc(sxba, 16)

        # DVE: constants
        nc.vector.memset(zero[:], 0.0).then_inc(svz, 1)
        nc.vector.memset(negz[:], -Z_CONST).then_inc(svz, 1)

        # PE stream
        for g in range(G):
            it, c = divmod(g, N_CHUNKS)
            for j in range(CHUNK // MM_N):
                if j == 0 and g >= 2:
                    nc.tensor.wait_ge(sact, g - 1)  # PSUM buf recycled
                blk = pt[g % 2][:, j * MM_N:(j + 1) * MM_N]
                if c == 0:
                    if g == 0 and j == 0:
                        nc.tensor.wait_ge(sya, 16)
                        nc.tensor.wait_ge(sxa, 16)
                    mm = nc.tensor.matmul(
                        blk,
                        xa[:, it * P:(it + 1) * P],
                        ya[:, j * MM_N:(j + 1) * MM_N],
                        start=True,
                        stop=True,
                    )
                else:
                    if g == 1 and j == 0:
                        for s in (syb, sxb, syba, sxba):
                            nc.tensor.wait_ge(s, 16)
                    nc.tensor.matmul(
                        blk,
                        xb[D:P, it * P:(it + 1) * P],
                        yb[D:P, j * MM_N:(j + 1) * MM_N],
                        start=True,
                        stop=False,
                    )
                    mm = nc.tensor.matmul(
                        blk,
                        xba[32:34, it * P:(it + 1) * P],
                        yba[32:34, j * MM_N:(j + 1) * MM_N],
                        start=False,
                        stop=True,
                    )
            mm.then_inc(spe, 1)

        # ACT stream: exp+accumulate per chunk, ln, -Z, result DMA
        nc.scalar.wait_ge(svz, 2)
        for g in range(G):
            nc.scalar.wait_ge(spe, g + 1)
            nc.scalar.activation(
                out=et[g % 2][:],
                in_=pt[g % 2][:],
                func=mybir.ActivationFunctionType.Exp,
                bias=zero[:],
                accum_out=sums[:, g:g + 1],
            ).then_inc(sact, 1)

        for it in range(T_TILES):
            nc.vector.wait_ge(sact, N_CHUNKS * (it + 1))
            r = nc.vector.reduce_sum(
                out=tots[:, it:it + 1],
                in_=sums[:, it * N_CHUNKS:(it + 1) * N_CHUNKS],
                axis=mybir.AxisListType.X,
            )
        r.then_inc(sv2, 1)

        nc.scalar.wait_ge(sv2, 1)
        nc.scalar.activation(
            out=lnt[:],
            in_=tots[:],
            func=mybir.ActivationFunctionType.Ln,
            bias=zero[:],
        )
        nc.scalar.activation(
            out=res[:],
            in_=lnt[:],
            func=mybir.ActivationFunctionType.Identity,
            bias=negz[:],
        )
        nc.scalar.dma_start(out=out_d[:], in_=res[:]).then_inc(so, 16)

        nc.vector.wait_ge(so, 16)
        sem_nums = sorted(s.num for s in my_sems)
        assert sem_nums == list(range(sem_nums[0], sem_nums[0] + len(sem_nums)))
        nc.vector.sem_clear(range(sem_nums[0], sem_nums[-1] + 1))

    nc.compile()
    _strip_preamble_and_merge_act_tables(nc)
    return nc


def _strip_preamble_and_merge_act_tables(nc):
    blk = nc.main_func.blocks[0]
    insts = list(blk.instructions)
    drop = set()
    for k, inst in enumerate(insts):
        tn = type(inst).__name__
        if tn == "InstEventSemaphore" and inst.name.startswith("barrier_"):
            drop.add(inst.name)  # boot-barrier event semaphores
            # ... and the per-engine drain feeding this barrier entry
            if k > 0 and type(insts[k - 1]).__name__ == "InstDrain":
                drop.add(insts[k - 1].name)
        elif tn == "InstMemset" and inst.outs and "const-" in str(inst.outs[0]):
            drop.add(inst.name)  # const-AP memsets (nothing reads the const APs)
    new_insts = []
    first_load_seen = False
    for inst in insts:
        if inst.name in drop:
            continue
        if type(inst).__name__ == "InstLoadActFuncSet":
            if first_load_seen:
                assert not inst.has_wait() and not inst.has_update(), inst.name
                continue
            inst.act_func_set_id = 6
            first_load_seen = True
        new_insts.append(inst)
    blk.instructions[:] = new_insts


_PROG = {}


def _get_prog(mm_dtype=MM_DTYPE, impl="packed"):
    key = (mm_dtype, impl)
    if key not in _PROG:
        builder = {
            "raw": build_program_raw,
            "packed": build_program_packed,
            "tile": build_program,
        }[impl]
        _PROG[key] = builder(mm_dtype)
    return _PROG[key]


def _prepare(test_Xs, train_Xs, weights):
    test_Xs = np.asarray(test_Xs, dtype=np.float32)
    train_Xs = np.asarray(train_Xs, dtype=np.float32)
    weights = np.asarray(weights, dtype=np.float32)

    test_sq = (test_Xs.astype(np.float64) ** 2).sum(1)
    train_sq = (train_Xs.astype(np.float64) ** 2).sum(1)
    scale = weights.astype(np.float64) ** 2

    xhat = np.empty((KA, NT), np.float32)
    xhat[:D] = test_Xs.T
    xhat[D] = test_sq
    xhat[D + 1] = 1.0

    yhat = np.empty((KA, NTR), np.float32)
    yhat[:D] = (train_Xs.astype(np.float64) * scale[:, None]).T
    yhat[D] = -0.5 * scale
    yhat[D + 1] = -0.5 * scale * train_sq
    return xhat, yhat


def kernel(test_Xs, train_Xs, weights, mm_dtype=MM_DTYPE, trace=False,
           impl="packed"):
    xhat, yhat = _prepare(test_Xs, train_Xs, weights)
    nc = _get_prog(mm_dtype, impl)
    if impl == "packed":
        ya = np.ascontiguousarray(yhat[:, :CHUNK])
        yb = np.ascontiguousarray(yhat[:D, CHUNK:])
        yba = np.ascontiguousarray(yhat[D:, CHUNK:])
        in_maps = []
        for c in range(N_CORES):
            xa = np.ascontiguousarray(xhat[:, c * TPC:(c + 1) * TPC])
            in_maps.append({
                "ya": ya, "yb": yb, "yba": yba,
                "xa": xa,
                "xb": np.ascontiguousarray(xa[:D]),
                "xba": np.ascontiguousarray(xa[D:]),
            })
    else:
        in_maps = [
            {"xhat": np.ascontiguousarray(xhat[:, c * TPC:(c + 1) * TPC]),
             "yhat": yhat}
            for c in range(N_CORES)
        ]
    res = run_bass_kernel_spmd(nc, in_maps, list(range(N_CORES)), trace=trace)
    parts = []
    for c in range(N_CORES):
        o = res.results[c]["out"]
        parts.append(o.T.ravel() if o.ndim == 2 else o)
    out = np.concatenate(parts)
    if trace:
        kernel.last_results = res
    return out



# revision 7
# speedup vs baseline: 1.6326x; 1.6326x over previous
"""Gaussian-KDE logsumexp kernel for Trainium2 (8 NeuronCores, SPMD).

Math: out[t] = ln( sum_n exp(a_tn) ) - Z,  a_tn = -0.5*scale_n*dist2[t,n].
One fp16 K=66 GEMM produces a_tn + K0 (the Schraudolph/exp shift K0 is folded
into the train-side augmented row):
    xhat[:, t] = [test_t (64), test_sq_t, 1]
    yhat[:, n] = [scale_n*train_n (64), -.5*scale_n, -.5*scale_n*train_sq_n + K0]
Per chunk the exp+reduce is split across engines:
  - ACT: exp table on cols [0:A) with bias -K0, free-dim accumulation.
  - DVE: Schraudolph fast exp on cols [A:2048): i32 = max(psum*C1, 0) is
    the float bit pattern of 2^((a+K0)/ln2 - 127) ~= exp(a); the bitcast-f32
    view is then reduce-summed.
Host does the final cross-core sum, ln, and -Z.

Sharding: 4 test-quarters x 2 train-halves = 8 cores; each core gets
512 test points (4 t-tiles of 128) x 2048 train points, all-fp16 inputs
(337 KB/core) DMA'd as one packed-f32 tensor: [x | y0] on the Scalar HWDGE
queue, [y1] on Sync. Partial sums [128, 8] go back over both queues; Sync
holds the NEFF exit open until the output-DMA acks land.
"""

import math
from contextlib import ExitStack

import numpy as np

import concourse.bacc as bacc
import concourse.mybir as mybir
from concourse.bass_utils import run_bass_kernel_spmd

N_CORES = 8
NT, NTR, D = 2048, 4096, 64
KA = D + 2                   # 66: augmented contraction dim
TW, TRW = 4, 2               # test ways x train ways (TW*TRW == 8)
TPC = NT // TW               # 512 test points per core
NPC = NTR // TRW             # 2048 train points per core
P = 128                      # partition tile of test points
T_TILES = TPC // P           # 4
MM_N = 512                   # matmul free-dim (one PSUM bank, fp32)
F32 = mybir.dt.float32
F16 = mybir.dt.float16
I32 = mybir.dt.int32

Z_CONST = float(0.5 * D * math.log(2.0 * math.pi) + math.log(NTR))  # h = 1
C1 = float(2.0 ** 23 / math.log(2.0))            # Schraudolph scale
K0 = float((127.0 - 0.0434609) * math.log(2.0))  # exp-arg shift (~88.0)

A_ACT = 1280                 # ACT exp columns per 2048-col chunk (rest: DVE)

# packed-f32 column counts of the combined [x | y] input tensor
XC = TPC // 2                # 256
YC = NPC // 2                # 1024
# DMA wave 1 = [x | y-half0] (cols 0:768), wave 2 = y-half1 (cols 768:1280)
W1 = XC + YC // 2            # 768


def build_program(impl="dve", a_act=A_ACT, warmup=0):
    """impl: 'act' (ACT does all exp), 'dve' (ACT+DVE split, DVE
    self-reduces its Schraudolph share)."""
    split = impl != "act"
    A = a_act if split else NPC
    B = NPC - A

    nc = bacc.Bacc("TRN2")
    xy = nc.declare_dram_parameter("xy", [KA, XC + YC], F32, isOutput=False)
    # two separate contiguous outputs: concurrent DMAs must never share a
    # 64B DRAM line (partial-line writes from different DMA engines race)
    out_a = nc.declare_dram_parameter("out_a", [P, T_TILES], F32, isOutput=True)
    out_b = nc.declare_dram_parameter("out_b", [P, T_TILES], F32, isOutput=True)

    with ExitStack() as ctx:
        sb = lambda nm, shape, dt: ctx.enter_context(nc.sbuf_tensor(nm, shape, dt))
        comb = sb("comb", [KA, XC + YC], F32)
        xs = comb[:, 0:XC].bitcast(F16)              # [66, 512]
        ys = comb[:, XC:XC + YC].bitcast(F16)        # [66, 2048]
        et = [sb(f"et{k}", [P, A], F32) for k in range(2)]
        ei = [sb(f"ei{k}", [P, max(B, 1)], I32) for k in range(2)]
        wake = sb("wake", [1, 1], F32)
        dummy_in = sb("dummy_in", [P, 1], F32)
        dummy_out = sb("dummy_out", [P, 1], F32)
        negk0 = sb("negk0", [P, 1], F32)
        sums = sb("sums", [P, 2 * T_TILES], F32)
        pt = [
            ctx.enter_context(nc.psum_tensor(f"pt{k}", [P, NPC], F32))
            for k in range(2)
        ]

        s1 = ctx.enter_context(nc.semaphore("s1"))
        s2 = ctx.enter_context(nc.semaphore("s2"))
        spe = ctx.enter_context(nc.semaphore("spe"))
        sact = ctx.enter_context(nc.semaphore("sact"))
        sdve = ctx.enter_context(nc.semaphore("sdve"))
        svz = ctx.enter_context(nc.semaphore("svz"))
        so = ctx.enter_context(nc.semaphore("so"))
        swk = ctx.enter_context(nc.semaphore("swk"))

        # --- Scalar (ACT, HWDGE): DGE wake, then the critical [x|y0] wave,
        # exp-table warm, exp chunks, and its half of the output
        nc.scalar.dma_start(out=wake[:], in_=xy[0:1, 0:1]).then_inc(swk, 16)
        nc.scalar.dma_start(out=comb[:, 0:W1], in_=xy[:, 0:W1]).then_inc(s1, 16)
        # dummy exp triggers the activation-table load at boot; bias AP is
        # garbage at this point, output unused
        nc.scalar.activation(
            dummy_out[:], dummy_in[:], mybir.ActivationFunctionType.Exp,
            bias=negk0[:],
        )

        # --- Sync (HWDGE): y-half1 wave
        nc.sync.dma_start(
            out=comb[:, W1:XC + YC], in_=xy[:, W1:XC + YC]
        ).then_inc(s2, 16)

        # --- DVE: constant
        nc.vector.memset(negk0[:], -K0).then_inc(svz, 1)

        # --- PE: optional p-state warmup on garbage, then the real stream
        for w in range(warmup):
            nc.tensor.matmul(
                pt[1][:, (w % 4) * MM_N:(w % 4 + 1) * MM_N],
                xs[:, 0:P],
                ys[:, 0:MM_N],
                start=True,
                stop=True,
            )
        for k in range(T_TILES):
            for j in range(NPC // MM_N):
                if k == 0 and j == 0:
                    nc.tensor.wait_ge(s1, 16)
                if k == 0 and j == 2:
                    nc.tensor.wait_ge(s2, 16)
                if k >= 2 and j == 0:
                    nc.tensor.wait_ge(sact, k - 1)
                    if split:
                        nc.tensor.wait_ge(sdve, k - 1)
                mm = nc.tensor.matmul(
                    pt[k % 2][:, j * MM_N:(j + 1) * MM_N],
                    xs[:, k * P:(k + 1) * P],
                    ys[:, j * MM_N:(j + 1) * MM_N],
                    start=True,
                    stop=True,
                )
            mm.then_inc(spe, 1)

        # --- ACT: exp + accumulate per chunk, then its half of the output
        nc.scalar.wait_ge(svz, 1)
        for k in range(T_TILES):
            nc.scalar.wait_ge(spe, k + 1)
            nc.scalar.activation(
                out=et[k % 2][:],
                in_=pt[k % 2][:, 0:A],
                func=mybir.ActivationFunctionType.Exp,
                bias=negk0[:],
                accum_out=sums[:, k:k + 1],
            ).then_inc(sact, 1)
        nc.scalar.dma_start(
            out=out_a[:], in_=sums[:, 0:T_TILES]
        ).then_inc(so, 16)

        out_acks = 16
        if split:
            # --- DVE: Schraudolph fast-exp + reduce of its own share
            for k in range(T_TILES):
                nc.vector.wait_ge(spe, k + 1)
                nc.vector.tensor_scalar(
                    out=ei[k % 2][:],
                    in0=pt[k % 2][:, A:NPC],
                    scalar1=C1,
                    scalar2=0.0,
                    op0=mybir.AluOpType.mult,
                    op1=mybir.AluOpType.max,
                ).then_inc(sdve, 1)
                r = nc.vector.reduce_sum(
                    out=sums[:, T_TILES + k:T_TILES + k + 1],
                    in_=ei[k % 2][:].bitcast(F32),
                    axis=mybir.AxisListType.X,
                )
            r.then_inc(sdve, 16)  # final value: T_TILES + 16

            # --- Sync ships the split half of the sums
            nc.sync.wait_ge(sdve, T_TILES + 16)
            nc.sync.dma_start(
                out=out_b[:],
                in_=sums[:, T_TILES:2 * T_TILES],
            ).then_inc(so, 16)
            out_acks = 32

        # --- Sync holds the NEFF exit open until the output DMAs acked
        nc.sync.wait_ge(so, out_acks)

    nc.compile()
    _strip_boot_barrier(nc)
    return nc


def _strip_boot_barrier(nc):
    """Drop the framework's all-engine boot barrier and const-AP memsets so
    every engine starts issuing immediately (mirrors the tuned baseline)."""
    blk = nc.main_func.blocks[0]
    insts = list(blk.instructions)
    drop = set()
    for i, inst in enumerate(insts):
        tn = type(inst).__name__
        if tn == "InstEventSemaphore" and inst.name.startswith("barrier_"):
            drop.add(inst.name)
            if i > 0 and type(insts[i - 1]).__name__ == "InstDrain":
                drop.add(insts[i - 1].name)
        elif tn == "InstMemset" and inst.outs and "const-" in str(inst.outs[0]):
            drop.add(inst.name)
    blk.instructions[:] = [i for i in insts if i.name not in drop]


_PROG = {}


def _get_prog(impl="dve", a_act=A_ACT, warmup=0):
    key = (impl, a_act, warmup)
    if key not in _PROG:
        _PROG[key] = build_program(impl, a_act, warmup)
    return _PROG[key]


def _prepare(test_Xs, train_Xs, weights):
    test_Xs = np.asarray(test_Xs, dtype=np.float32)
    train_Xs = np.asarray(train_Xs, dtype=np.float32)
    weights = np.asarray(weights, dtype=np.float32)

    test_sq = (test_Xs.astype(np.float64) ** 2).sum(1)
    train_sq = (train_Xs.astype(np.float64) ** 2).sum(1)
    scale = weights.astype(np.float64) ** 2

    xhat = np.empty((KA, NT), np.float16)
    xhat[:D] = test_Xs.T
    xhat[D] = test_sq
    xhat[D + 1] = 1.0

    yhat = np.empty((KA, NTR), np.float16)
    yhat[:D] = (train_Xs.astype(np.float64) * scale[:, None]).T
    yhat[D] = -0.5 * scale
    yhat[D + 1] = -0.5 * scale * train_sq + K0
    return xhat, yhat


def kernel(test_Xs, train_Xs, weights, impl="dve", a_act=A_ACT, warmup=0,
           trace=False):
    xhat, yhat = _prepare(test_Xs, train_Xs, weights)
    nc = _get_prog(impl, a_act, warmup)
    in_maps = []
    for c in range(N_CORES):
        i, j = c >> 1, c & 1
        xy = np.empty((KA, 2 * (XC + YC)), np.float16)
        xy[:, 0:TPC] = xhat[:, i * TPC:(i + 1) * TPC]
        xy[:, TPC:] = yhat[:, j * NPC:(j + 1) * NPC]
        in_maps.append({"xy": xy.view(np.float32)})
    res = run_bass_kernel_spmd(nc, in_maps, list(range(N_CORES)), trace=trace)

    S = np.zeros(NT, np.float64)
    for c in range(N_CORES):
        i = c >> 1
        part = res.results[c]["out_a"].astype(np.float64)    # [128, 4]
        if impl != "act":
            part = part + res.results[c]["out_b"].astype(np.float64)
        # t = i*TPC + k*P + p  <-> column-major flatten of part[p, k]
        S[i * TPC:(i + 1) * TPC] += part.T.ravel()
    out = (np.log(S) - Z_CONST).astype(np.float32)
    if trace:
        kernel.last_results = res
    return out
